# revision 28
# baseline (speedup 1.0000x reference)
"""MultiHeadedAttention Trainium2 kernel (8-core SPMD, data-parallel).

Sharding: 8 cores = (batch b in 0..3) x (query half in 0..1). Each core
computes out[b, half*1024:(half+1)*1024, :] independently - no collectives.

Quarter-interleaved schedule: k/v projections are emitted per 512-row
quarter with head-pair 0's attention j-tiles right behind the chunks that
feed them, so the ScalarE exp stream starts ~60us into the kernel. x
staging is a single DRAM->DRAM SWDGE cast per chunk; mask casts on GpSimd;
R = bo + bv@Wo^T precomputed on host; xattnT overlays qT's SBUF; finalize
via bf16 Z-row PE-broadcast + reciprocal (HW-verified primitives only).
"""
import numpy as np
import ml_dtypes

import concourse.bass as bass
import concourse.mybir as mybir
import concourse.tile as tile
from concourse import bacc
from concourse.bass_utils import run_bass_kernel_spmd

F32 = mybir.dt.float32
BF16 = mybir.dt.bfloat16
I32 = mybir.dt.int32
AF = mybir.ActivationFunctionType
ALU = mybir.AluOpType

N_CORES = 8
DK = 64


def slices(total, chunk):
    return [(s, min(chunk, total - s)) for s in range(0, total, chunk)]


class Cfg:
    def __init__(self, SQ=1024, SK=2048, DM=1024, H=16, max_stage=5):
        assert DM % 128 == 0 and SK % 512 == 0 and SQ % 512 == 0 and H % 2 == 0
        self.SQ, self.SK, self.DM, self.H = SQ, SK, DM, H
        self.KT = DM // 128          # dm contraction chunks
        self.HP = H // 2             # head pairs
        self.NJ = SK // 128          # Sk tiles
        self.NQK = SK // 512         # sk quarters
        self.NQQ = SQ // 512         # sq quarters
        self.SQS = min(1024, SQ)     # attention Sq slice width (2 psum banks)
        self.max_stage = max_stage
        assert SQ % self.SQS == 0
        assert H * DK == DM


def emit_kernel(tc, cfg, io):
    nc = tc.nc
    C = cfg
    xq, xk, xv, msk = io["xq"], io["xk"], io["xv"], io["mask"]
    w_dram = {"q": io["wqt"], "k": io["wkt"], "v": io["wvt"], "o": io["wot"]}
    bql, bkl, rrow = io["bql"], io["bkl"], io["rrow"]
    out = io["out"]

    pools = {}

    def open_pool(name, bufs=1, space="SBUF"):
        pools[name] = tc.alloc_tile_pool(name=name, bufs=bufs, space=space)
        return pools[name]

    def close_pool(name):
        pools[name].release()
        del pools[name]

    persist = open_pool("persist", 1)
    dram = open_pool("dram", 1, space="DRAM")
    ps_s = open_pool("ps_s", 2, space="PSUM")
    ps_pv = open_pool("ps_pv", 2, space="PSUM")

    # ---------------- persistent tiles ----------------
    # qT_sb doubles as xattnT: finalize(hp) writes the hp column range only
    # after the last scores read of qT(hp)
    qT_sb = persist.tile([128, C.HP * C.SQ], BF16, name="qT_sb")
    kT_sb = persist.tile([128, C.HP * C.SK], BF16, name="kT_sb")
    v_sb = persist.tile([128, C.NJ * C.H * 65], BF16, name="v_sb")
    maskT_sb = persist.tile([128, C.NJ * C.SQ], BF16, name="maskT_sb")
    bql_sb = persist.tile([128, C.HP], F32, name="bql_sb")
    bkl_sb = persist.tile([128, C.HP], F32, name="bkl_sb")
    rrow_sb = persist.tile([128, C.DM], F32, name="rrow_sb")
    onesb_sb = persist.tile([1, 128], BF16, name="onesb_sb")
    xattnT_sb = qT_sb

    nc.sync.dma_start(bql_sb[:], bql[:])
    nc.sync.dma_start(bkl_sb[:], bkl[:])
    nc.sync.dma_start(rrow_sb[:], rrow[:])
    nc.vector.memset(onesb_sb[:], 1.0)

    v_view = v_sb.rearrange("p (j h c) -> p j h c", j=C.NJ, c=65)
    # ones column per head (Z accumulator row of the PV matmul)
    nc.vector.memset(v_view[:, :, :, 64:65], 1.0)

    PS_F = max(C.SQS, 512)

    # ---------------- weights ----------------
    wkv_pool = open_pool("wkv", 1)
    mask_stg = open_pool("mask_stg", 1)
    wq_pool = open_pool("wq", 1)
    wq_sb = wq_pool.tile([128, C.KT * C.DM], BF16, name="w_q")
    wk_sb = wkv_pool.tile([128, C.KT * C.DM], BF16, name="w_k")
    wv_sb = wkv_pool.tile([128, C.KT * C.DM], BF16, name="w_v")
    # wq on the SP queue (ahead of the q transposes), wk/wv on the ACT queue
    for kt in range(C.KT):
        nc.sync.dma_start(wq_sb[:, kt * C.DM:(kt + 1) * C.DM],
                          w_dram["q"][kt * 128:(kt + 1) * 128, :])
    for nm, t in (("k", wk_sb), ("v", wv_sb)):
        for kt in range(C.KT):
            nc.scalar.dma_start(t[:, kt * C.DM:(kt + 1) * C.DM],
                                w_dram[nm][kt * 128:(kt + 1) * 128, :])

    # ---------------- mask staging (per 512-col chunk, lazily) -----------
    mstg = dram.tile([C.SQ, C.SK], BF16, name="mstg")

    def stage_mask_chunk(qq):
        cs = qq * 512
        nst = C.SQ // 128
        tis = []
        for st in range(nst):
            ti = mask_stg.tile([128, 512], I32, name="mint", tag="mint",
                               bufs=3, padded_shape=[128, 512])
            nc.gpsimd.dma_start(ti[:], msk[st * 128:(st + 1) * 128, cs:cs + 512])
            tis.append(ti)
        for st in range(nst):
            tb = mask_stg.tile([128, 512], BF16, name="mbf", tag="mbf",
                               bufs=2, padded_shape=[128, 512])
            nc.gpsimd.tensor_copy(tb[:], tis[st][:])
            nc.gpsimd.dma_start(mstg[st * 128:(st + 1) * 128, cs:cs + 512], tb[:])

    def transpose_mask_chunk(qq):
        for j in range(qq * 4, qq * 4 + 4):
            nc.scalar.dma_start(
                maskT_sb[:, j * C.SQ:(j + 1) * C.SQ],
                mstg[:, j * 128:(j + 1) * 128],
                transpose=True,
            )

    # ---------------- x staging: one-hop DRAM->DRAM casts ----------------
    stg = {}

    def stage1_x(name, x_in, r0, r1):
        if name not in stg:
            stg[name] = dram.tile([x_in.shape[0], C.DM], BF16,
                                  name=f"stg_{name}", uniquify=True)
        nc.gpsimd.dma_start(stg[name][r0:r1, :], x_in[r0:r1, :])

    def load_xT_quarter(name, qq, dst):
        """transpose rows [qq*512, qq*512+512) of stg into dst[128, KT*512]"""
        for kt in range(C.KT):
            nc.sync.dma_start(
                dst[:, kt * 512:(kt + 1) * 512],
                stg[name][qq * 512:(qq + 1) * 512, kt * 128:(kt + 1) * 128],
                transpose=True,
            )

    # ---------------- Q staging + projection ----------------
    stage1_x("q", xq, 0, C.SQ)
    stage1_x("k", xk, 0, 512)
    stage1_x("v", xv, 0, 512)
    stage_mask_chunk(0)

    xq_pool = open_pool("xq", 1)
    for qq in range(C.NQQ):
        xq_c = xq_pool.tile([128, C.KT * 512], BF16, name="xq_c", tag="xq",
                            bufs=2)
        load_xT_quarter("q", qq, xq_c)
        for hp in range(C.HP):
            ps = ps_s.tile([128, 512], F32, name="ps_qp", tag="s",
                           padded_shape=[128, PS_F])
            for kt in range(C.KT):
                nc.tensor.matmul(
                    ps[:],
                    wq_sb[:, kt * C.DM + hp * 128: kt * C.DM + (hp + 1) * 128],
                    xq_c[:, kt * 512:(kt + 1) * 512],
                    start=(kt == 0), stop=(kt == C.KT - 1),
                )
            nc.scalar.activation(
                qT_sb[:, hp * C.SQ + qq * 512: hp * C.SQ + qq * 512 + 512],
                ps[:], AF.Identity, bias=bql_sb[:, hp:hp + 1])
    close_pool("xq")
    close_pool("wq")

    # ---------------- attention machinery ----------------
    attn = open_pool("attn", 1)

    def finish():
        for pl in reversed(list(pools.values())):
            pl.release()

    state = {"pv": None, "hist": []}
    PIPE = 3

    def attn_begin(hp):
        state["pv"] = [
            ps_pv.tile([65, C.SQS], F32, name=f"ps_pv{i}", tag="pv",
                       padded_shape=[65, PS_F])
            for i in range(2)
        ]
        state["hist"] = []

    def emit_pv(jj, pms, hp):
        pv = state["pv"]
        for i in range(2):
            for (qs, qw) in slices(C.SQS, 512):
                nc.tensor.matmul(
                    pv[i][:, qs:qs + qw], v_view[:, jj, 2 * hp + i, :],
                    pms[i][:, qs:qs + qw],
                    start=(jj == 0), stop=(jj == C.NJ - 1),
                )

    def attn_j(hp, j):
        sss = [ps_s.tile([128, C.SQS], F32, name=f"ps_sc{i}", tag="s",
                         padded_shape=[128, PS_F]) for i in range(2)]
        for (qs, qw) in slices(C.SQS, 512):
            for i in range(2):
                nc.tensor.matmul(
                    sss[i][:, qs:qs + qw],
                    kT_sb[i * 64:(i + 1) * 64,
                          hp * C.SK + j * 128: hp * C.SK + (j + 1) * 128],
                    qT_sb[i * 64:(i + 1) * 64,
                          hp * C.SQ + qs: hp * C.SQ + qs + qw],
                    start=True, stop=True,
                )
        pms = []
        for i in range(2):
            pe = attn.tile([128, C.SQS], BF16, name="p_exp", tag="pexp",
                           bufs=2, padded_shape=[128, C.SQS])
            nc.scalar.activation(pe[:], sss[i][:], AF.Exp)
            pm = attn.tile([128, C.SQS], BF16, name="p_msk", tag="pmask",
                           bufs=6, padded_shape=[128, C.SQS])
            nc.vector.tensor_tensor(
                out=pm[:], in0=pe[:],
                in1=maskT_sb[:, j * C.SQ: j * C.SQ + C.SQS],
                op=ALU.mult,
            )
            pms.append(pm)
        state["hist"].append((j, pms))
        if len(state["hist"]) > PIPE:
            jj, pp = state["hist"].pop(0)
            emit_pv(jj, pp, hp)

    def attn_end(hp):
        for jj, pp in state["hist"]:
            emit_pv(jj, pp, hp)
        state["hist"] = []
        pv = state["pv"]
        for i in range(2):
            # Z row -> bf16 sbuf; PE-broadcast (bf16); reciprocal; multiply
            zrow = attn.tile([1, C.SQS], BF16, name="zrow", tag="zrow", bufs=1,
                             padded_shape=[1, C.SQS])
            nc.vector.tensor_copy(zrow[:], pv[i][64:65, :])
            zb = ps_s.tile([64, C.SQS], F32, name="zb", tag="s",
                           padded_shape=[128, PS_F])
            for (qs, qw) in slices(C.SQS, 512):
                nc.tensor.matmul(zb[:, qs:qs + qw], onesb_sb[0:1, 0:64],
                                 zrow[0:1, qs:qs + qw], start=True, stop=True)
            zr = attn.tile([64, C.SQS], F32, name="zr", tag="zr", bufs=1,
                           padded_shape=[64, C.SQS])
            nc.vector.reciprocal_approx_fast(out=zr[:], in_=zb[:])
            tmp = attn.tile([64, C.SQS], BF16, name="xat_t", tag="xat_t",
                            bufs=2, padded_shape=[64, C.SQS])
            nc.vector.tensor_tensor(out=tmp[:], in0=pv[i][0:64, :],
                                    in1=zr[:], op=ALU.mult)
            nc.sync.dma_start(
                xattnT_sb[64 * i:64 * (i + 1), hp * C.SQ: hp * C.SQ + C.SQS],
                tmp[:],
            )

    # ---------------- quarter loop: k/v proj + attention(hp0) ----------
    xkv_pool = open_pool("xkv", 1)
    attn_begin(0)
    for qq in range(C.NQK):
        if qq + 1 < C.NQK:   # prefetch next quarter's staging
            stage1_x("k", xk, (qq + 1) * 512, (qq + 2) * 512)
            stage1_x("v", xv, (qq + 1) * 512, (qq + 2) * 512)
            stage_mask_chunk(qq + 1)
        xk_c = xkv_pool.tile([128, C.KT * 512], BF16, name="xk_c", tag="xk",
                             bufs=2)
        load_xT_quarter("k", qq, xk_c)
        xv_c = xkv_pool.tile([128, C.KT * 512], BF16, name="xv_c", tag="xv",
                             bufs=1)
        load_xT_quarter("v", qq, xv_c)
        # K projection for this quarter, all head pairs
        for hp in range(C.HP):
            ps = ps_s.tile([128, 512], F32, name="ps_kp", tag="s",
                           padded_shape=[128, PS_F])
            for kt in range(C.KT):
                nc.tensor.matmul(
                    ps[:],
                    wk_sb[:, kt * C.DM + hp * 128: kt * C.DM + (hp + 1) * 128],
                    xk_c[:, kt * 512:(kt + 1) * 512],
                    start=(kt == 0), stop=(kt == C.KT - 1),
                )
            nc.scalar.activation(
                kT_sb[:, hp * C.SK + qq * 512: hp * C.SK + qq * 512 + 512],
                ps[:], AF.Identity, bias=bkl_sb[:, hp:hp + 1])
        # V projection for this quarter's 4 j-tiles
        for jt in range(4):
            j = qq * 4 + jt
            ps = ps_s.tile([128, C.DM], F32, name="ps_v", tag="s",
                           padded_shape=[128, PS_F])
            for (ds_, dw) in slices(C.DM, 512):
                for kt in range(C.KT):
                    nc.tensor.matmul(
                        ps[:, ds_:ds_ + dw],
                        xv_c[:, kt * 512 + jt * 128: kt * 512 + (jt + 1) * 128],
                        wv_sb[:, kt * C.DM + ds_: kt * C.DM + ds_ + dw],
                        start=(kt == 0), stop=(kt == C.KT - 1),
                    )
            for (ds_, dw) in slices(C.DM, 512):
                hs, hw = ds_ // DK, dw // DK
                nc.vector.tensor_copy(
                    v_view[:, j, hs:hs + hw, 0:64],
                    ps[:, ds_:ds_ + dw].rearrange("p (h c) -> p h c", c=DK),
                )
        # attention head-pair 0 over this quarter's j tiles
        transpose_mask_chunk(qq)
        for jt in range(4):
            attn_j(0, qq * 4 + jt)
    attn_end(0)

    if C.max_stage <= 2:
        finish()
        return

    # ---------------- remaining head pairs ----------------
    for hp in range(1, C.HP):
        attn_begin(hp)
        for j in range(C.NJ):
            attn_j(hp, j)
        attn_end(hp)
    close_pool("xkv")
    close_pool("attn")
    close_pool("mask_stg")
    close_pool("wkv")

    if C.max_stage <= 3:
        finish()
        return

    # ---------------- output projection ----------------
    wo_pool = open_pool("wo", 1)
    wo_sb = wo_pool.tile([128, C.KT * C.DM], BF16, name="w_o")
    for kt in range(C.KT):
        nc.scalar.dma_start(wo_sb[:, kt * C.DM:(kt + 1) * C.DM],
                            w_dram["o"][kt * 128:(kt + 1) * 128, :])
    epi = open_pool("epi", 1)
    for m in range(C.SQ // 128):
        ps = ps_pv.tile([128, C.DM], F32, name="ps_o", tag="pv",
                        padded_shape=[128, PS_F])
        for (qs, qw) in slices(C.DM, 512):
            for hp in range(C.HP):
                nc.tensor.matmul(
                    ps[:, qs:qs + qw],
                    xattnT_sb[:, hp * C.SQ + m * 128: hp * C.SQ + (m + 1) * 128],
                    wo_sb[:, hp * C.DM + qs: hp * C.DM + qs + qw],
                    start=(hp == 0), stop=(hp == C.HP - 1),
                )
        ot = epi.tile([128, C.DM], F32, name="out_sb", tag="out_sb", bufs=2,
                      padded_shape=[128, PS_F])
        nc.vector.tensor_tensor(out=ot[:], in0=ps[:], in1=rrow_sb[:],
                                op=ALU.add)
        nc.sync.dma_start(out[m * 128:(m + 1) * 128, :], ot[:])

    finish()


def build(cfg, reps=1):
    nc = bacc.Bacc("TRN2", target_bir_lowering=False, debug=False)
    C = cfg
    io = {
        "xq": nc.dram_tensor("xq", [C.SQ, C.DM], F32, kind="ExternalInput").ap(),
        "xk": nc.dram_tensor("xk", [C.SK, C.DM], F32, kind="ExternalInput").ap(),
        "xv": nc.dram_tensor("xv", [C.SK, C.DM], F32, kind="ExternalInput").ap(),
        "mask": nc.dram_tensor("mask", [C.SQ, C.SK], I32, kind="ExternalInput").ap(),
        "wqt": nc.dram_tensor("wqt", [C.DM, C.DM], BF16, kind="ExternalInput").ap(),
        "wkt": nc.dram_tensor("wkt", [C.DM, C.DM], BF16, kind="ExternalInput").ap(),
        "wvt": nc.dram_tensor("wvt", [C.DM, C.DM], BF16, kind="ExternalInput").ap(),
        "wot": nc.dram_tensor("wot", [C.DM, C.DM], BF16, kind="ExternalInput").ap(),
        "bql": nc.dram_tensor("bql", [128, C.HP], F32, kind="ExternalInput").ap(),
        "bkl": nc.dram_tensor("bkl", [128, C.HP], F32, kind="ExternalInput").ap(),
        "rrow": nc.dram_tensor("rrow", [128, C.DM], F32, kind="ExternalInput").ap(),
        "out": nc.dram_tensor("out", [C.SQ, C.DM], F32, kind="ExternalOutput").ap(),
    }
    with tile.TileContext(nc) as tc:
        for _ in range(reps):
            emit_kernel(tc, cfg, io)
    nc.compile()
    return nc


def host_prep(query, key, value, mask, Wq, bq, Wk, bk, Wv, bv, Wo, bo, cfg):
    """Host-side layout prep (weight transpose/cast, R row, per-core slicing)."""
    C = cfg
    bf = ml_dtypes.bfloat16
    wqt = np.ascontiguousarray((Wq.T * 0.125).astype(bf))   # 1/sqrt(dk) folded
    wkt = np.ascontiguousarray(Wk.T.astype(bf))
    wvt = np.ascontiguousarray(Wv.T.astype(bf))
    wot = np.ascontiguousarray(Wo.T.astype(bf))
    bql = np.ascontiguousarray((bq * 0.125).reshape(C.HP, 128).T.astype(np.float32))
    bkl = np.ascontiguousarray(bk.reshape(C.HP, 128).T.astype(np.float32))
    rrow = np.ascontiguousarray(np.broadcast_to(
        (bo + bv @ Wo.T).reshape(1, C.DM), (128, C.DM)).astype(np.float32))
    shared = dict(wqt=wqt, wkt=wkt, wvt=wvt, wot=wot, bql=bql, bkl=bkl,
                  rrow=rrow)
    in_maps = []
    B = query.shape[0]
    halves = query.shape[1] // C.SQ
    for c in range(B * halves):
        b, h = divmod(c, halves)
        m = dict(shared)
        m["xq"] = np.ascontiguousarray(query[b, h * C.SQ:(h + 1) * C.SQ, :])
        m["xk"] = np.ascontiguousarray(key[b])
        m["xv"] = np.ascontiguousarray(value[b])
        m["mask"] = np.ascontiguousarray(mask[b, h * C.SQ:(h + 1) * C.SQ, :])
        in_maps.append(m)
    return in_maps


_CACHED = {}


def get_built():
    if "nc" not in _CACHED:
        _CACHED["nc"] = build(Cfg())
    return _CACHED["nc"]


def kernel(query, key, value, mask, Wq, bq, Wk, bk, Wv, bv, Wo, bo):
    cfg = Cfg()
    nc = get_built()
    in_maps = host_prep(query, key, value, mask, Wq, bq, Wk, bk, Wv, bv, Wo, bo, cfg)
    res = run_bass_kernel_spmd(nc, in_maps, core_ids=list(range(N_CORES)))
    B, S, DM = query.shape
    out = np.empty((B, S, DM), np.float32)
    for c in range(N_CORES):
        b, h = divmod(c, 2)
        out[b, h * cfg.SQ:(h + 1) * cfg.SQ, :] = res.results[c]["out"]
    return out
